# revision 35
# baseline (speedup 1.0000x reference)
"""AdaptiveGraphLearner distributed Trainium2 kernel (8 NeuronCores), v9.

reference:  sim = (x @ x.T)/0.1;  adj = sim * rowwise_top32_mask(sim)
            out = (adj + adj.T)/2
Identity (sim symmetric):  out[a,b] = h[a,b] * ([h[a,b] > t33_a] + [h[a,b] > cmid_b])
with h = 0.5*sim, t33_a = 33rd largest of row a, cmid_b = (e32_b+e33_b)/2.

Single-matmul-pass design: the whole h row block is compressed to int16
"q-space" at eviction time and everything downstream (threshold scan, the
collective exchange, mask compares, and even the output values) runs on the
compressed form, so h is never recomputed and never stored in fp32:

- q(h) = sat_i16(rne(Lrelu(24*h - 8640, alpha=8.5))): piecewise-linear
  monotone map with slope 204/h-unit below the knee at h=360 (where all
  top-33 thresholds live: bucket 0.005 h-units vs mean e32/e33 gap 1.06)
  and 24/h-unit above (value bucket 0.042, used only for output values).
  Very negative h saturates to -32768 (always below thresholds: harmless).
  One ScalarE activation per PSUM quarter produces it directly.
- Threshold scan (hierarchical max8: top-8 of 32 chunks of 256, then 4
  rounds of max8+match_replace) runs on the i16 q tiles; thresholds are
  plain q values: row q33 (exact bitwise ties) and col floor((q32+q33)/2).
- Per-row-block 256B AllGathers of the i16 column thresholds (8 pipelined
  collectives, ~15us each on this mesh; one 4KB AllGather costs ~125us).
  Threshold DMAs ride the GpSimd queue so the AllGather-dependent qcb
  scatters never head-of-line block the Sync queue.
- Mask phase (no matmuls): per half row block, row mask via i16
  tensor_scalar (4x DVE mode), col mask via i16 tensor_tensor (2x), add
  (2x), then values are dequantized from q by two ScalarE activations
  (Lrelu(q, 1/8.5) then affine to bf16) and multiplied by the mask on DVE
  (all-16-bit: 2x). bf16 output, host upconverts to fp32.
- DVE and GpSimd share one SBUF port pair (lock per instruction), so all
  mask passes stay on DVE; ScalarE/PE have their own SBUF ports.
- fp16 x fp16 matmuls (host converts x): one-sided fp16 noise dominates the
  error; host sim == HW: rel err 1.3845e-2 vs the 2e-2 gate.
"""
import sys
sys.path.insert(0, '/opt/trn_rl_repo')
import numpy as np
import concourse.bass as bass
import concourse.bacc as bacc
import concourse.mybir as mybir
import concourse.tile as tile
from concourse.bass_utils import run_bass_kernel_spmd

N, DIM, K = 8192, 256, 32
TEMP = 0.1
SCALE = 0.5 / TEMP
NCORES = 8
RPC = N // NCORES          # rows per core
NB = RPC // 128            # 8 row blocks of 128
QW = 2048                  # psum tile width (4 banks)
NQ = N // QW               # 4 quarters per row block
NCHUNK = 32                # threshold scan chunks
CHUNK = N // NCHUNK        # 256

QSC = 24.0                 # q slope above the knee (values)
QAL = 8.0                  # extra slope factor below the knee (integer: the HW alpha field truncates fractions)
QKNEE = 360.0              # knee position in h units
QBIAS = -QSC * QKNEE       # -8640

f32 = mybir.dt.float32
f16 = mybir.dt.float16
bf16 = mybir.dt.bfloat16
i16 = mybir.dt.int16
COPY = mybir.ActivationFunctionType.Copy
PRELU = mybir.ActivationFunctionType.Prelu  # Lrelu ignores alpha on this HW (fixed 0.01); Prelu honors it
GT = mybir.AluOpType.is_gt
ADD = mybir.AluOpType.add
MUL = mybir.AluOpType.mult


def build_nc():
    nc = bacc.Bacc(None, target_bir_lowering=False, num_devices=NCORES)
    xT = nc.declare_dram_parameter("xT", [DIM, N], f16, isOutput=False)
    xgT = nc.declare_dram_parameter("xgT", [DIM, RPC], f16, isOutput=False)
    out = nc.declare_dram_parameter("out", [RPC, N], bf16, isOutput=True)

    with tile.TileContext(nc) as tc:
        with tc.tile_pool(name="dram", bufs=1, space="DRAM") as dram:
            t_locs = [dram.tile([128], i16, name=f"t_loc{k}") for k in range(NB)]
            t_alls = [dram.tile([NCORES * 128], i16, addr_space="Shared",
                                name=f"t_all{k}") for k in range(NB)]

            with tc.tile_pool(name="keep", bufs=1) as keep_pool:
                # row thresholds as fp32 holding the exact i16 value (the
                # DVE tensor_scalar is_gt path requires an fp32 scalar)
                qt33f = keep_pool.tile([128, NB], f32, name="qt33f")
                # qcb[p, i, c] = col threshold for global column i*1024+c
                qcb = keep_pool.tile([128, NCORES, RPC], i16, name="qcb")
                qbias = keep_pool.tile([128, 1], f32, name="qbias")
                b360 = keep_pool.tile([128, 1], f32, name="b360")
                nc.vector.memset(qbias[:], float(QBIAS))
                nc.vector.memset(b360[:], float(QKNEE))
                # compressed h, one persistent tile per row block
                qts = [keep_pool.tile([128, N], i16, name=f"q{k}")
                       for k in range(NB)]

                with tc.tile_pool(name="xin", bufs=1) as xin_pool, \
                     tc.tile_pool(name="ps", bufs=2, space="PSUM") as ps_pool, \
                     tc.tile_pool(name="thr", bufs=1) as thr_pool:
                    xr0 = xin_pool.tile([128, N], f16, name="xr0")
                    xr1 = xin_pool.tile([128, N], f16, name="xr1")
                    xgr0 = xin_pool.tile([128, RPC], f16, name="xgr0")
                    xgr1 = xin_pool.tile([128, RPC], f16, name="xgr1")
                    nc.sync.dma_start(xr0[:], xT[0:128, :])
                    nc.sync.dma_start(xr1[:], xT[128:256, :])
                    nc.sync.dma_start(xgr0[:], xgT[0:128, :])
                    nc.sync.dma_start(xgr1[:], xgT[128:256, :])

                    # ---- Phase 1: matmuls -> q tiles -> thresholds -> AG --
                    for rb in range(NB):
                        qt = qts[rb]
                        r0, r1 = rb * 128, (rb + 1) * 128
                        for q in range(NQ):
                            c0 = q * QW
                            p = ps_pool.tile([128, QW], f32, name="p", tag="p")
                            for ct in range(4):
                                s0, s1 = ct * 512, (ct + 1) * 512
                                nc.tensor.matmul(p[:, s0:s1], xgr0[:, r0:r1],
                                                 xr0[:, c0 + s0:c0 + s1],
                                                 start=True, stop=False)
                            for ct in range(4):
                                s0, s1 = ct * 512, (ct + 1) * 512
                                nc.tensor.matmul(p[:, s0:s1], xgr1[:, r0:r1],
                                                 xr1[:, c0 + s0:c0 + s1],
                                                 start=False, stop=True)
                            nc.scalar.activation(qt[:, c0:c0 + QW], p[:],
                                                 PRELU, bias=qbias[:],
                                                 scale=float(QSC * SCALE),
                                                 alpha=float(QAL))
                        cand16 = thr_pool.tile([128, NCHUNK * 8], i16,
                                               name="cand16", tag="cand16")
                        for c in range(NCHUNK):
                            nc.vector.max(out=cand16[:, c * 8:(c + 1) * 8],
                                          in_=qt[:, c * CHUNK:(c + 1) * CHUNK])
                        # match_replace rejects i16; run the selection rounds
                        # on an fp32 copy of the 256 candidates (values are
                        # integers so the conversion is exact)
                        cand = thr_pool.tile([128, NCHUNK * 8], f32,
                                             name="cand", tag="cand")
                        nc.vector.tensor_copy(cand[:], cand16[:])
                        m8x = thr_pool.tile([128, 16], f32, name="m8x",
                                            tag="m8x")
                        m8a, m8b = m8x[:, 0:8], m8x[:, 8:16]
                        tf = thr_pool.tile([128, 1], f32, name="tf", tag="tf")
                        qmid = thr_pool.tile([128, 1], i16, name="qmid",
                                             tag="qmid")
                        for r in range(4):
                            nc.vector.max(out=m8a, in_=cand[:])
                            nc.vector.match_replace(out=cand[:],
                                                    in_to_replace=m8a,
                                                    in_values=cand[:],
                                                    imm_value=-1e30)
                        nc.vector.max(out=m8b, in_=cand[:])
                        # row threshold q33 as exact fp32
                        nc.vector.tensor_copy(qt33f[:, rb:rb + 1],
                                              m8b[:, 0:1])
                        # column threshold floor((q32+q33)/2)
                        nc.vector.tensor_add(tf[:, 0:1], m8a[:, 7:8],
                                             qt33f[:, rb:rb + 1])
                        nc.vector.tensor_scalar(
                            out=tf[:, 0:1], in0=tf[:, 0:1],
                            scalar1=0.5, scalar2=-0.499999,
                            op0=MUL, op1=ADD)
                        nc.vector.tensor_copy(qmid[:], tf[:, 0:1])
                        # gpsimd queue keeps Sync free of AG-dependent waits
                        nc.gpsimd.dma_start(t_locs[rb][:], qmid[:])
                        nc.gpsimd.collective_compute(
                            "AllGather", mybir.AluOpType.bypass,
                            replica_groups=[list(range(NCORES))],
                            ins=[t_locs[rb].opt()], outs=[t_alls[rb].opt()])
                        src = (t_alls[rb].tensor.reshape([1, NCORES, 128]).ap()
                               .to_broadcast((128, NCORES, 128)))
                        nc.sync.dma_start(
                            qcb[:, :, rb * 128:(rb + 1) * 128], src)

                # ---- Phase 2: dequant + mask (no matmuls) -----------------
                HW = N // 2
                with tc.tile_pool(name="mp", bufs=1) as mp_pool, \
                     tc.tile_pool(name="pv", bufs=1) as pv_pool, \
                     tc.tile_pool(name="hv", bufs=2) as hv_pool:
                    for rb in range(NB):
                        qt = qts[rb]
                        r0, r1 = rb * 128, (rb + 1) * 128
                        qtf = qt33f[:, rb:rb + 1]
                        for s0, s1 in ((0, HW), (HW, N)):
                            # dequant values on ScalarE (own ports):
                            # hv = 360 + Lrelu(q, 1/8.5)/24, as bf16
                            pv = pv_pool.tile([128, HW], f32, name="pv",
                                              tag="pv")
                            nc.scalar.activation(pv[:], qt[:, s0:s1], PRELU,
                                                 scale=-1.0,
                                                 alpha=float(QAL))
                            hv = hv_pool.tile([128, HW], bf16, name="hv",
                                              tag="hv")
                            # dequantized values are always >= ~199 so Relu
                            # is a no-op; Copy would reject the AP bias
                            nc.scalar.activation(hv[:], pv[:],
                                                 mybir.ActivationFunctionType.Relu,
                                                 bias=b360[:],
                                                 scale=float(-1.0 / (QAL * QSC)))
                            # masks on DVE (i16/u16 fast modes)
                            mr = mp_pool.tile([128, HW], i16, name="mr",
                                              tag="mr")
                            m = mp_pool.tile([128, HW], i16, name="m",
                                             tag="m")
                            nc.vector.tensor_scalar(
                                out=mr[:], in0=qt[:, s0:s1],
                                scalar1=qtf, scalar2=None, op0=GT)
                            nc.vector.tensor_tensor(
                                out=m[:], in0=qt[:, s0:s1],
                                in1=qcb[:, s0 // RPC:s1 // RPC, :], op=GT)
                            nc.vector.tensor_tensor(out=m[:], in0=m[:],
                                                    in1=mr[:], op=ADD)
                            nc.vector.tensor_tensor(out=hv[:], in0=hv[:],
                                                    in1=m[:], op=MUL)
                            nc.sync.dma_start(out[r0:r1, s0:s1], hv[:])

    nc.compile()
    return nc


_nc_cache = None


def get_nc():
    global _nc_cache
    if _nc_cache is None:
        _nc_cache = build_nc()
    return _nc_cache


def kernel_with_result(x, trace: bool = False):
    x = np.ascontiguousarray(np.asarray(x), dtype=np.float32)
    assert x.shape == (N, DIM)
    nc = get_nc()
    xT = np.ascontiguousarray(x.T.astype(np.float16))
    in_maps = []
    for i in range(NCORES):
        xg = np.ascontiguousarray(
            x[i * RPC:(i + 1) * RPC, :].T.astype(np.float16))
        in_maps.append({"xT": xT, "xgT": xg})
    res = run_bass_kernel_spmd(nc, in_maps, core_ids=list(range(NCORES)),
                               trace=trace)
    outp = np.concatenate(
        [np.asarray(res.results[i]["out"]).astype(np.float32)
         for i in range(NCORES)], axis=0)
    return outp, res


def kernel(x) -> np.ndarray:
    outp, _res = kernel_with_result(x)
    return outp
